# revision 19
# baseline (speedup 1.0000x reference)
"""DistMult bilinear scoring kernel for Trainium2 (8 NeuronCores).

scores[e] = left_emb[e] @ W[r_id[e]] @ right_emb[e]

The problem is HBM-bandwidth bound (E=4.2M edges x 32 dims x 2 tensors).
Strategy:
  Host: stable-sort edges by relation (data-parallel shard over 8 cores),
        pad each relation bucket to 512-edge chunks, cast L/R to fp16
        (halves HBM traffic; ~3e-4 rel err) and pre-layout both into the
        transposed block format the PE wants (dim on partitions), packed
        L|R per unit so each unit is ONE 2 MB DMA. The relation of every
        512-edge chunk is baked into the compiled kernel as a static
        weight-slice schedule. A ragged tail unit avoids rounding the
        per-core edge count up to a full 16384-edge unit.
  Device (identical program on all 8 cores), per 16384-edge unit:
    - DMA in packed L|R fp16 [128, 8192]
    - per 512-wide quarter: one block-diagonal fp16 matmul
      (W[r] per 32-row block) -> V.T in PSUM (fp32)
    - DVE: Z = V.T (*) Rt elementwise -> fp16 (two quarters per op)
    - PE: 4 shifted block-ones fp16 matmuls accumulate each half-unit's
      block sums into a DENSE [16, 512] PSUM tile (8192 scores)
    - ACT: cast-copy scores PSUM->SBUF fp16; one 512 KB DMA out per
      16-unit score group
  Host: inverse-permute scores back to the original edge order (fp32).

HBM traffic per core: ~67.6 MB in + ~1 MB out (vs 139+16.5 MB for fp32
with 8x-sparse score output) -> ~2.2x faster at the ~358 GB/s roofline.
"""

import math
import os
import sys

import numpy as np

for _p in ("/opt/trn_rl_repo", "/root/.axon_site/_ro/trn_rl_repo"):
    if os.path.isdir(_p) and _p not in sys.path:
        sys.path.insert(0, _p)
        break

import concourse.bass as bass
import concourse.mybir as mybir
import concourse.tile as tile
from concourse import bacc, bass_utils

F32 = mybir.dt.float32
F16 = mybir.dt.float16

DIM = 32
NUM_REL = 8
N_CORES = 8
CHUNK = 512                      # edges per (pb, quarter) cell
N_Q = 8                          # quarters per full unit
TILE_FREE = N_Q * CHUNK          # 4096: free dim of a full unit (per L or R)
TILE_E = 4 * TILE_FREE           # 16384 edges per full unit
EPQ = 4 * CHUNK                  # 2048 edges per quarter
GROUP_HALVES = 32                # half-units per score-output DMA group
SCORE_COLS = 4 * CHUNK           # 2048 cols in a score group tile

_module_cache = {}
LAST_RESULTS = None  # BassKernelResults of the most recent run (for test.py)
_hooks_installed = False
_ldw_patched = False


def _patch_walrus_ldw_opt():
    """Enable walrus LDWEIGHTS dedup: consecutive matmuls with identical
    stationary operands skip the reload, which otherwise serializes ~220 ns
    per matmul on the PE queue."""
    global _ldw_patched
    if _ldw_patched:
        return
    _ldw_patched = True
    orig = bass_utils.run_command

    def patched(argv, **kw):
        argv = [
            "--enable-ldw-opt=true" if a == "--enable-ldw-opt=false" else a
            for a in argv
        ]
        return orig(argv, **kw)

    bass_utils.run_command = patched


def _ensure_profiling_hooks():
    """Make trace=True work in this container: install the NTFF profile hook
    (ctypes into libaxon_pjrt.so, same ABI trn_boot uses) and no-op the S3
    artifact upload."""
    global _hooks_installed
    if _hooks_installed:
        return
    _hooks_installed = True
    bass_utils.upload_artifacts = lambda tmpdir: str(tmpdir)
    try:
        import antenv.axon_hooks  # noqa: F401

        return
    except ImportError:
        pass
    import contextlib
    import ctypes
    import types

    hook = None
    so_path = "/opt/axon/libaxon_pjrt.so"
    if os.path.exists(so_path):
        lib = ctypes.CDLL(so_path)
        if hasattr(lib, "axon_start_nrt_profile"):
            lib.axon_start_nrt_profile.argtypes = [
                ctypes.POINTER(ctypes.c_int64),
                ctypes.c_size_t,
            ]
            lib.axon_start_nrt_profile.restype = ctypes.c_int64
            lib.axon_stop_nrt_profile.argtypes = [ctypes.c_char_p]
            lib.axon_stop_nrt_profile.restype = ctypes.c_int64

            @contextlib.contextmanager
            def _hook(output_dir, device_ids):
                import jax

                jax.devices()
                if device_ids:
                    ids = (ctypes.c_int64 * len(device_ids))(*device_ids)
                    rc = lib.axon_start_nrt_profile(ids, len(device_ids))
                else:
                    rc = lib.axon_start_nrt_profile(None, 0)
                if rc != 0:
                    raise RuntimeError(f"axon_start_nrt_profile rc={rc}")
                try:
                    yield
                finally:
                    n = lib.axon_stop_nrt_profile(str(output_dir).encode())
                    print(f"profile: {n} ntff file(s) in {output_dir}", file=sys.stderr)

            hook = _hook

    mod = types.ModuleType("antenv.axon_hooks")
    mod._hook = hook
    mod.get_axon_ntff_profile_hook = lambda: mod._hook

    def _set(h):
        mod._hook = h

    mod.set_axon_ntff_profile_hook = _set
    import antenv

    sys.modules["antenv.axon_hooks"] = mod
    antenv.axon_hooks = mod


def _unit_list(n_chunks: int):
    """[(n_q per unit)] covering n_chunks 512-chunks: full 8-quarter units
    plus one ragged tail unit (1..8 quarters)."""
    n_full = n_chunks // (4 * N_Q)
    rem = n_chunks - n_full * 4 * N_Q
    q_t = (rem + 3) // 4
    units = [N_Q] * n_full
    if q_t:
        units.append(q_t)
    return units, n_full, q_t


def _quarter_variants(units, rel_sched):
    """Variant (block-diagonal W combo) of each (unit, quarter).

    Quarter h2 of unit u covers chunks base_u + n_q*pb + h2 for pb=0..3,
    which form the 4 diagonal 32x32 blocks of its stationary matrix."""
    combos = []
    combo_idx = {}
    var_of = []
    base = 0
    for n_q in units:
        row = []
        for h2 in range(n_q):
            c = tuple(rel_sched[base + n_q * pb + h2] for pb in range(4))
            if c not in combo_idx:
                combo_idx[c] = len(combos)
                combos.append(c)
            row.append(combo_idx[c])
        var_of.append(row)
        base += 4 * n_q
    return combos, var_of


def _build_module(units, rel_sched):
    """Build the single-core Bass program (same program runs on all 8 cores).

    Note: the walrus LDW-dedup pass (--enable-ldw-opt=true) rejects the
    accumulating reduce matmuls here; fp16 stationaries get hardware Fast
    Weight Load automatically, so the dedup isn't needed (unlike f32r)."""
    nc = bacc.Bacc(None, target_bir_lowering=False)
    combos, var_of = _quarter_variants(units, rel_sched)
    n_var = len(combos)
    n_full = sum(1 for q in units if q == N_Q)
    q_t = 0 if len(units) == n_full else units[-1]

    # total half-unit groups; pairs of halves share one [32, 512] score tile
    halves = []  # (unit_idx, hf, [quarter indices])
    for u, n_q in enumerate(units):
        for hf in range((n_q + 3) // 4):
            qs = list(range(4 * hf, min(4 * hf + 4, n_q)))
            halves.append((u, hf, qs))
    n_halves = len(halves)
    n_pairs = (n_halves + 1) // 2
    n_og = math.ceil(n_halves / GROUP_HALVES)

    lr_d = None
    if n_full:
        lr_d = nc.dram_tensor(
            "lr", (2 * n_full, 128, TILE_FREE), F16, kind="ExternalInput"
        )
    lrt_d = None
    if q_t:
        lrt_d = nc.dram_tensor(
            "lrt", (128, 2 * q_t * CHUNK), F16, kind="ExternalInput"
        )
    w_d = nc.dram_tensor("wdiag", (128, n_var * 128), F16, kind="ExternalInput")
    o_d = nc.dram_tensor("ones32", (128, 256), F16, kind="ExternalInput")
    s_d = nc.dram_tensor(
        "scores", (n_og, 128, SCORE_COLS), F16, kind="ExternalOutput"
    )

    with tile.TileContext(nc) as tc:
        with (
            tc.tile_pool(name="const", bufs=1) as cpool,
            tc.tile_pool(name="io", bufs=8) as iop,
            tc.tile_pool(name="zp", bufs=12) as zp,
            tc.tile_pool(name="sp", bufs=2) as sp,
            tc.tile_pool(name="vsb", bufs=4) as vbp,
            tc.tile_pool(name="vps", bufs=3, space="PSUM") as vpool,
            tc.tile_pool(name="sps", bufs=2, space="PSUM") as spool,
        ):
            wdiag = cpool.tile([128, n_var * 128], F16, name="wdiag_sb")
            nc.sync.dma_start(wdiag[:], w_d[:])
            ones32 = cpool.tile([128, 256], F16, name="ones32_sb")
            nc.sync.dma_start(ones32[:], o_d[:])

            state = {"s_sbuf": None}

            def flush(pair, hp):
                # pair: list of (z_parts, parity); z_parts =
                # [(z_tile, z_col0, j0, nq_in_part)] covering a half's
                # quarters. Reduce the pair to a dense [32,512] (even half
                # in rows 0..15, odd in 16..31) and cast-copy to SBUF.
                pp = hp % (GROUP_HALVES // 2)
                if pp == 0:
                    state["s_sbuf"] = sp.tile(
                        [128, SCORE_COLS], F16, tag="s", name="s_sb"
                    )
                s_sbuf = state["s_sbuf"]
                s_ps = spool.tile([32, CHUNK], F32, tag="sps", name="s_ps")
                n_mm = sum(p[3] for zp_, par in pair for p in zp_)
                mi = 0
                for z_parts, par in pair:
                    for z, z_col0, j0, nq in z_parts:
                        for t in range(nq):
                            s = 4 * par + j0 + t
                            nc.tensor.matmul(
                                s_ps[0:32, :],
                                ones32[:, 32 * s : 32 * s + 32],
                                z[:, z_col0 + CHUNK * t : z_col0 + CHUNK * (t + 1)],
                                start=(mi == 0),
                                stop=(mi == n_mm - 1),
                                tile_position=(0, 0),
                            )
                            mi += 1
                # dense pair scores -> fp16 SBUF at (32*(pp%4), 512*(pp//4))
                nc.scalar.copy(
                    s_sbuf[
                        32 * (pp % 4) : 32 * (pp % 4) + 32,
                        CHUNK * (pp // 4) : CHUNK * (pp // 4) + CHUNK,
                    ],
                    s_ps[0:32, :],
                )
                if pp == GROUP_HALVES // 2 - 1 or hp == n_pairs - 1:
                    go = hp // (GROUP_HALVES // 2)
                    cols = CHUNK * ((pp // 4) + 1)
                    nc.scalar.dma_start(s_d[go, :, 0:cols], s_sbuf[:, 0:cols])

            pending = []  # complete pairs awaiting flush
            cur_pair = []  # halves accumulated toward the current pair
            hp_idx = 0
            h_idx = 0
            for u, n_q in enumerate(units):
                if n_q != N_Q:
                    lrt = iop.tile(
                        [128, 2 * n_q * CHUNK], F16, tag="lrt", name="lrt_sb"
                    )
                    nc.sync.dma_start(lrt[:], lrt_d[:])

                for hf in range((n_q + 3) // 4):
                    qs = list(range(4 * hf, min(4 * hf + 4, n_q)))
                    if n_q == N_Q:
                        # one 1 MB DMA per half-unit: [L_half | R_half]
                        fcols = 4 * CHUNK
                        q0 = 4 * hf
                        lr = iop.tile(
                            [128, 2 * fcols], F16, tag="lrh", name="lrh_sb"
                        )
                        nc.sync.dma_start(lr[:], lr_d[2 * u + hf])
                    else:
                        fcols = n_q * CHUNK
                        q0 = 0
                        lr = lrt
                    # V = block-diag W @ L, two quarters per PSUM tile
                    vps = []  # (vp_tile, [quarters], col0_in_lr)
                    for g0 in range(0, len(qs), 2):
                        gq = qs[g0 : g0 + 2]
                        vp = vpool.tile(
                            [128, CHUNK * len(gq)], F32, tag=f"v{len(gq)}", name="v_ps"
                        )
                        for t, h2 in enumerate(gq):
                            nc.tensor.matmul(
                                vp[:, CHUNK * t : CHUNK * (t + 1)],
                                wdiag[:, 128 * var_of[u][h2] : 128 * (var_of[u][h2] + 1)],
                                lr[:, CHUNK * (h2 - q0) : CHUNK * (h2 - q0 + 1)],
                                tile_position=(0, 0),
                            )
                        vps.append((vp, gq))

                    if len(pending) > 1:
                        flush(pending.pop(0), hp_idx)
                        hp_idx += 1

                    # Z = V (*) R elementwise, fp16. DVE's 1x PSUM path is
                    # the kernel's scarcest resource, so alternate: pair A
                    # multiplies straight from PSUM (DVE 1x); pair B is
                    # cast-copied PSUM->SBUF by ACT (which sits next to
                    # PSUM), letting DVE run it in 2x fp16 SBUF mode.
                    z_parts = []
                    for idx, (vp, gq) in enumerate(vps):
                        w = CHUNK * len(gq)
                        src_v = vp
                        if idx % 2 == 1:
                            vsb = vbp.tile(
                                [128, w], F16, tag=f"vs{len(gq)}", name="v_sb"
                            )
                            nc.scalar.copy(vsb[:], vp[:])
                            src_v = vsb
                        z = zp.tile([128, w], F16, tag=f"z{len(gq)}", name="z_sb")
                        nc.vector.tensor_tensor(
                            z[:],
                            src_v[:],
                            lr[:, fcols + CHUNK * (gq[0] - q0) : fcols + CHUNK * (gq[0] - q0) + w],
                            op=mybir.AluOpType.mult,
                        )
                        z_parts.append((z, 0, gq[0] - 4 * hf, len(gq)))
                    cur_pair.append((z_parts, h_idx % 2))
                    h_idx += 1
                    if len(cur_pair) == 2:
                        pending.append(cur_pair)
                        cur_pair = []

            if cur_pair:
                pending.append(cur_pair)
            for p in pending:
                flush(p, hp_idx)
                hp_idx += 1
    nc.finalize()
    return nc


def _relayout(X, n_q):
    """[2048*n_q, 32] edge-major -> [128, 512*n_q] PE block layout:
    [32*pb + k, 512*h2 + n] = X[512*n_q*pb + 512*h2 + n, k]."""
    return (
        X.reshape(4, n_q, CHUNK, DIM).transpose(0, 3, 1, 2).reshape(128, n_q * CHUNK)
    )


def _prep_inputs(left, right, rid):
    """Sort/pad/shard/relayout on the host. Returns device arrays + recovery info."""
    E = left.shape[0]
    perm = np.argsort(rid, kind="stable")
    counts = np.bincount(rid, minlength=NUM_REL).astype(np.int64)
    starts = np.zeros(NUM_REL + 1, dtype=np.int64)
    np.cumsum(counts, out=starts[1:])

    # per-core segment length per relation, multiple of CHUNK
    seg = [
        int(math.ceil(c / (N_CORES * CHUNK))) * CHUNK if c > 0 else 0 for c in counts
    ]
    per_core_real = int(sum(seg))
    n_chunks = max(1, per_core_real // CHUNK)
    units, n_full, q_t = _unit_list(n_chunks)
    T = 512 * 4 * sum(units)  # padded per-core slot count

    # relation schedule of each sorted 512-chunk (identical on every core)
    rel_sched = []
    for r in range(NUM_REL):
        rel_sched += [r] * (seg[r] // CHUNK)
    rel_sched += [0] * (T // CHUNK - len(rel_sched))

    # gather index (into sorted order) for each device slot; -1 = padding
    gidx = np.full((N_CORES, T), -1, dtype=np.int64)
    off = 0
    for r in range(NUM_REL):
        s = seg[r]
        if s == 0:
            continue
        ar = np.arange(s, dtype=np.int64)
        for c in range(N_CORES):
            src = c * s + ar
            gidx[c, off : off + s] = np.where(src < counts[r], starts[r] + src, -1)
        off += s

    L_s = left[perm]
    R_s = right[perm]

    LR = (
        np.zeros((N_CORES, 2 * n_full, 128, TILE_FREE), np.float16)
        if n_full
        else None
    )
    LRT = (
        np.zeros((N_CORES, 128, 2 * q_t * CHUNK), np.float16) if q_t else None
    )
    for c in range(N_CORES):
        gi = gidx[c]
        msk = gi >= 0
        Lc = np.zeros((T, DIM), np.float32)
        Rc = np.zeros((T, DIM), np.float32)
        Lc[msk] = L_s[gi[msk]]
        Rc[msk] = R_s[gi[msk]]
        Lc = Lc.astype(np.float16)
        Rc = Rc.astype(np.float16)
        full_e = n_full * TILE_E
        if n_full:
            # per half-unit [L_half | R_half]; pb stride stays unit-wide:
            # half hf of unit u covers [pb, 4hf+lq, n, k]
            Lf = Lc[:full_e].reshape(n_full, 4, N_Q, CHUNK, DIM)
            Rf = Rc[:full_e].reshape(n_full, 4, N_Q, CHUNK, DIM)
            for u in range(n_full):
                for hf in range(2):
                    sl = slice(4 * hf, 4 * hf + 4)
                    LR[c, 2 * u + hf, :, : TILE_FREE // 2] = (
                        Lf[u, :, sl].transpose(0, 3, 1, 2).reshape(128, 2048)
                    )
                    LR[c, 2 * u + hf, :, TILE_FREE // 2 :] = (
                        Rf[u, :, sl].transpose(0, 3, 1, 2).reshape(128, 2048)
                    )
        if q_t:
            LRT[c, :, : q_t * CHUNK] = _relayout(Lc[full_e:], q_t)
            LRT[c, :, q_t * CHUNK :] = _relayout(Rc[full_e:], q_t)
    return perm, gidx, units, tuple(rel_sched), LR, LRT


def _recover_scores(results, perm, gidx, units, E):
    n_full = sum(1 for q in units if q == N_Q)
    q_t = 0 if len(units) == n_full else units[-1]
    n_halves = 2 * n_full + ((q_t + 3) // 4 if q_t else 0)
    n_og = math.ceil(n_halves / GROUP_HALVES)
    T = 512 * 4 * sum(units)

    scores_sorted = np.zeros(E, np.float32)
    for c in range(N_CORES):
        sc = np.asarray(results[c]["scores"], dtype=np.float32)
        # [og, 32*(pp%4) + 16*par + r, 512*(pp//4) + n];
        # half = 2*(4*colblk + pblk4) + par
        sc = (
            sc.reshape(n_og, 4, 2, 16, 4, CHUNK)
            .transpose(0, 4, 1, 2, 3, 5)
            .reshape(n_og * GROUP_HALVES, 16, CHUNK)[:n_halves]
        )
        out = np.empty(T, np.float32)
        if n_full:
            # half h=2u+hf; row r=4j+b; pos = 16384u + 4096b + 2048hf + 512j + n
            out[: n_full * TILE_E] = (
                sc[: 2 * n_full]
                .reshape(n_full, 2, 4, 4, CHUNK)
                .transpose(0, 3, 1, 2, 4)
                .reshape(n_full * TILE_E)
            )
        if q_t:
            base = n_full * TILE_E
            for t in range((q_t + 3) // 4):
                half = sc[2 * n_full + t]
                for j in range(min(4, q_t - 4 * t)):
                    for b in range(4):
                        pos = base + CHUNK * (q_t * b + 4 * t + j)
                        out[pos : pos + CHUNK] = half[4 * j + b]
        gi = gidx[c]
        msk = gi >= 0
        scores_sorted[gi[msk]] = out[msk]
    scores = np.empty(E, np.float32)
    scores[perm] = scores_sorted
    return scores


def kernel(left_emb, right_emb, r_id, W):
    global LAST_RESULTS
    left = np.ascontiguousarray(np.asarray(left_emb, dtype=np.float32))
    right = np.ascontiguousarray(np.asarray(right_emb, dtype=np.float32))
    rid = np.asarray(r_id).astype(np.int64)
    Wn = np.asarray(W, dtype=np.float32)
    E = left.shape[0]

    perm, gidx, units, rel_sched, LR, LRT = _prep_inputs(left, right, rid)

    # block-diagonal stationary-W variants, one per distinct quarter combo
    combos, _ = _quarter_variants(units, rel_sched)
    wdiag3 = np.zeros((len(combos), 128, 128), np.float32)
    for v, combo in enumerate(combos):
        for pb, r in enumerate(combo):
            wdiag3[v, 32 * pb : 32 * pb + 32, 32 * pb : 32 * pb + 32] = Wn[r]
    # packed [128, n_var*128] so the whole table loads in one contiguous DMA
    wdiag = np.ascontiguousarray(
        wdiag3.transpose(1, 0, 2).reshape(128, len(combos) * 128)
    ).astype(np.float16)
    # shifted block-ones: slice s=4p+j sums 32-row block b of quarter j
    # into dense row 16p + 4j + b of the [32,512] pair-score tile
    ones32 = np.zeros((128, 256), np.float16)
    for p in range(2):
        for j in range(4):
            for b in range(4):
                ones32[32 * b : 32 * b + 32, 32 * (4 * p + j) + 16 * p + 4 * j + b] = 1.0

    key = (tuple(units), rel_sched)
    if key not in _module_cache:
        _module_cache.clear()
        _module_cache[key] = _build_module(list(units), rel_sched)
    nc = _module_cache[key]

    in_maps = []
    for c in range(N_CORES):
        m = {"wdiag": wdiag, "ones32": ones32}
        if LR is not None:
            m["lr"] = LR[c]
        if LRT is not None:
            m["lrt"] = LRT[c]
        in_maps.append(m)
    trace = bool(int(os.environ.get("KERNEL_TRACE", "0")))
    kwargs = {}
    if trace:
        _ensure_profiling_hooks()
        tdir = os.environ.get("KERNEL_TRACE_DIR")
        if tdir:
            os.makedirs(tdir, exist_ok=True)
            kwargs["tmpdir"] = tdir
    res = bass_utils.run_bass_kernel_spmd(
        nc, in_maps, core_ids=list(range(N_CORES)), trace=trace, **kwargs
    )
    LAST_RESULTS = res
    return _recover_scores(res.results, perm, gidx, units, E)


# revision 21
# speedup vs baseline: 1.2536x; 1.2536x over previous
"""DistMult bilinear scoring kernel for Trainium2 (8 NeuronCores).

scores[e] = left_emb[e] @ W[r_id[e]] @ right_emb[e]

The problem is HBM-bandwidth bound (E=4.2M edges x 32 dims x 2 tensors).
Strategy:
  Host: stable-sort edges by relation (data-parallel shard over 8 cores),
        pad each relation bucket to 512-edge chunks, cast L/R to fp16
        (halves HBM traffic; ~3e-4 rel err) and pre-layout both into the
        transposed block format the PE wants (dim on partitions), packed
        L|R per unit so each unit is ONE 2 MB DMA. The relation of every
        512-edge chunk is baked into the compiled kernel as a static
        weight-slice schedule. A ragged tail unit avoids rounding the
        per-core edge count up to a full 16384-edge unit.
  Device (identical program on all 8 cores), per 16384-edge unit:
    - DMA in packed L|R fp16 [128, 8192] (one 2 MB transfer; measured
      fastest vs 1 MB-per-half or 4 MB-per-two-units granularity)
    - per 512-wide quarter: one block-diagonal fp16 matmul
      (W[r] per 32-row block) -> V.T in PSUM (fp32)
    - Z = V.T (*) Rt elementwise -> fp16. DVE's PSUM read path is 1x
      mode (the kernel's scarcest resource), so per half-unit: pair A
      multiplies straight from PSUM (DVE 1x), pair B is cast-copied
      PSUM->SBUF by ACT (which sits next to PSUM), letting DVE run it
      in 2x fp16 SBUF mode.
    - PE: 8 parity/position-shifted block-ones fp16 matmuls accumulate
      each PAIR of half-units' block sums into a DENSE [32, 512] PSUM
      tile (16384 scores; engine APs need 32-aligned partition bases,
      which is why halves are paired)
    - ACT: cast-copy scores PSUM->SBUF fp16; one 512 KB DMA out per
      16-unit score group
  Host: inverse-permute scores back to the original edge order (fp32).

HBM traffic per core: ~67.6 MB in + ~1 MB out (vs 139+16.5 MB for fp32
with 8x-sparse score output) -> ~2.3x faster at the ~360 GB/s per-core
HBM roofline. Measured 193-220 us vs 440-463 us for the fp32 baseline
(run-to-run machine variance is ~10%).
"""

import math
import os
import sys

import numpy as np

for _p in ("/opt/trn_rl_repo", "/root/.axon_site/_ro/trn_rl_repo"):
    if os.path.isdir(_p) and _p not in sys.path:
        sys.path.insert(0, _p)
        break

import concourse.bass as bass
import concourse.mybir as mybir
import concourse.tile as tile
from concourse import bacc, bass_utils

F32 = mybir.dt.float32
F16 = mybir.dt.float16

DIM = 32
NUM_REL = 8
N_CORES = 8
CHUNK = 512                      # edges per (pb, quarter) cell
N_Q = 8                          # quarters per full unit
TILE_FREE = N_Q * CHUNK          # 4096: free dim of a full unit (per L or R)
TILE_E = 4 * TILE_FREE           # 16384 edges per full unit
EPQ = 4 * CHUNK                  # 2048 edges per quarter
GROUP_HALVES = 32                # half-units per score-output DMA group
SCORE_COLS = 4 * CHUNK           # 2048 cols in a score group tile

_module_cache = {}
LAST_RESULTS = None  # BassKernelResults of the most recent run (for test.py)
_hooks_installed = False


def _ensure_profiling_hooks():
    """Make trace=True work in this container: install the NTFF profile hook
    (ctypes into libaxon_pjrt.so, same ABI trn_boot uses) and no-op the S3
    artifact upload."""
    global _hooks_installed
    if _hooks_installed:
        return
    _hooks_installed = True
    bass_utils.upload_artifacts = lambda tmpdir: str(tmpdir)
    try:
        import antenv.axon_hooks  # noqa: F401

        return
    except ImportError:
        pass
    import contextlib
    import ctypes
    import types

    hook = None
    so_path = "/opt/axon/libaxon_pjrt.so"
    if os.path.exists(so_path):
        lib = ctypes.CDLL(so_path)
        if hasattr(lib, "axon_start_nrt_profile"):
            lib.axon_start_nrt_profile.argtypes = [
                ctypes.POINTER(ctypes.c_int64),
                ctypes.c_size_t,
            ]
            lib.axon_start_nrt_profile.restype = ctypes.c_int64
            lib.axon_stop_nrt_profile.argtypes = [ctypes.c_char_p]
            lib.axon_stop_nrt_profile.restype = ctypes.c_int64

            @contextlib.contextmanager
            def _hook(output_dir, device_ids):
                import jax

                jax.devices()
                if device_ids:
                    ids = (ctypes.c_int64 * len(device_ids))(*device_ids)
                    rc = lib.axon_start_nrt_profile(ids, len(device_ids))
                else:
                    rc = lib.axon_start_nrt_profile(None, 0)
                if rc != 0:
                    raise RuntimeError(f"axon_start_nrt_profile rc={rc}")
                try:
                    yield
                finally:
                    n = lib.axon_stop_nrt_profile(str(output_dir).encode())
                    print(f"profile: {n} ntff file(s) in {output_dir}", file=sys.stderr)

            hook = _hook

    mod = types.ModuleType("antenv.axon_hooks")
    mod._hook = hook
    mod.get_axon_ntff_profile_hook = lambda: mod._hook

    def _set(h):
        mod._hook = h

    mod.set_axon_ntff_profile_hook = _set
    import antenv

    sys.modules["antenv.axon_hooks"] = mod
    antenv.axon_hooks = mod


def _unit_list(n_chunks: int):
    """[(n_q per unit)] covering n_chunks 512-chunks: full 8-quarter units
    plus one ragged tail unit (1..8 quarters)."""
    n_full = n_chunks // (4 * N_Q)
    rem = n_chunks - n_full * 4 * N_Q
    q_t = (rem + 3) // 4
    units = [N_Q] * n_full
    if q_t:
        units.append(q_t)
    return units, n_full, q_t


def _quarter_variants(units, rel_sched):
    """Variant (block-diagonal W combo) of each (unit, quarter).

    Quarter h2 of unit u covers chunks base_u + n_q*pb + h2 for pb=0..3,
    which form the 4 diagonal 32x32 blocks of its stationary matrix."""
    combos = []
    combo_idx = {}
    var_of = []
    base = 0
    for n_q in units:
        row = []
        for h2 in range(n_q):
            c = tuple(rel_sched[base + n_q * pb + h2] for pb in range(4))
            if c not in combo_idx:
                combo_idx[c] = len(combos)
                combos.append(c)
            row.append(combo_idx[c])
        var_of.append(row)
        base += 4 * n_q
    return combos, var_of


def _build_module(units, rel_sched):
    """Build the single-core Bass program (same program runs on all 8 cores).

    Note: the walrus LDW-dedup pass (--enable-ldw-opt=true) rejects the
    accumulating reduce matmuls here; fp16 stationaries get hardware Fast
    Weight Load automatically, so the dedup isn't needed (unlike f32r)."""
    nc = bacc.Bacc(None, target_bir_lowering=False)
    combos, var_of = _quarter_variants(units, rel_sched)
    n_var = len(combos)
    n_full = sum(1 for q in units if q == N_Q)
    q_t = 0 if len(units) == n_full else units[-1]

    # total half-unit groups; pairs of halves share one [32, 512] score tile
    halves = []  # (unit_idx, hf, [quarter indices])
    for u, n_q in enumerate(units):
        for hf in range((n_q + 3) // 4):
            qs = list(range(4 * hf, min(4 * hf + 4, n_q)))
            halves.append((u, hf, qs))
    n_halves = len(halves)
    n_pairs = (n_halves + 1) // 2
    n_og = math.ceil(n_halves / GROUP_HALVES)

    lr_d = None
    if n_full:
        lr_d = nc.dram_tensor(
            "lr", (n_full, 128, 2 * TILE_FREE), F16, kind="ExternalInput"
        )
    lrt_d = None
    if q_t:
        lrt_d = nc.dram_tensor(
            "lrt", (128, 2 * q_t * CHUNK), F16, kind="ExternalInput"
        )
    w_d = nc.dram_tensor("wdiag", (128, n_var * 128), F16, kind="ExternalInput")
    o_d = nc.dram_tensor("ones32", (128, 256), F16, kind="ExternalInput")
    s_d = nc.dram_tensor(
        "scores", (n_og, 128, SCORE_COLS), F16, kind="ExternalOutput"
    )

    with tile.TileContext(nc) as tc:
        with (
            tc.tile_pool(name="const", bufs=1) as cpool,
            tc.tile_pool(name="io", bufs=6) as iop,
            tc.tile_pool(name="zp", bufs=10) as zp,
            tc.tile_pool(name="sp", bufs=2) as sp,
            tc.tile_pool(name="vsb", bufs=4) as vbp,
            tc.tile_pool(name="vps", bufs=3, space="PSUM") as vpool,
            tc.tile_pool(name="sps", bufs=2, space="PSUM") as spool,
        ):
            wdiag = cpool.tile([128, n_var * 128], F16, name="wdiag_sb")
            nc.sync.dma_start(wdiag[:], w_d[:])
            ones32 = cpool.tile([128, 256], F16, name="ones32_sb")
            nc.sync.dma_start(ones32[:], o_d[:])

            state = {"s_sbuf": None}

            def flush(pair, hp):
                # pair: list of (z_parts, parity); z_parts =
                # [(z_tile, z_col0, j0, nq_in_part)] covering a half's
                # quarters. Reduce the pair to a dense [32,512] (even half
                # in rows 0..15, odd in 16..31) and cast-copy to SBUF.
                pp = hp % (GROUP_HALVES // 2)
                if pp == 0:
                    state["s_sbuf"] = sp.tile(
                        [128, SCORE_COLS], F16, tag="s", name="s_sb"
                    )
                s_sbuf = state["s_sbuf"]
                s_ps = spool.tile([32, CHUNK], F32, tag="sps", name="s_ps")
                n_mm = sum(p[3] for zp_, par in pair for p in zp_)
                mi = 0
                for z_parts, par in pair:
                    for z, z_col0, j0, nq in z_parts:
                        for t in range(nq):
                            s = 4 * par + j0 + t
                            nc.tensor.matmul(
                                s_ps[0:32, :],
                                ones32[:, 32 * s : 32 * s + 32],
                                z[:, z_col0 + CHUNK * t : z_col0 + CHUNK * (t + 1)],
                                start=(mi == 0),
                                stop=(mi == n_mm - 1),
                                tile_position=(0, 0),
                            )
                            mi += 1
                # dense pair scores -> fp16 SBUF at (32*(pp%4), 512*(pp//4))
                nc.scalar.copy(
                    s_sbuf[
                        32 * (pp % 4) : 32 * (pp % 4) + 32,
                        CHUNK * (pp // 4) : CHUNK * (pp // 4) + CHUNK,
                    ],
                    s_ps[0:32, :],
                )
                if pp == GROUP_HALVES // 2 - 1 or hp == n_pairs - 1:
                    go = hp // (GROUP_HALVES // 2)
                    cols = CHUNK * ((pp // 4) + 1)
                    nc.scalar.dma_start(s_d[go, :, 0:cols], s_sbuf[:, 0:cols])

            pending = []  # complete pairs awaiting flush
            cur_pair = []  # halves accumulated toward the current pair
            hp_idx = 0
            h_idx = 0
            for u, n_q in enumerate(units):
                fcols = n_q * CHUNK
                lr = iop.tile(
                    [128, 2 * fcols], F16, tag=f"lr{n_q}", name=f"lr_sb{n_q}"
                )
                if n_q == N_Q:
                    nc.sync.dma_start(lr[:], lr_d[u])
                else:
                    nc.sync.dma_start(lr[:], lrt_d[:])

                for hf in range((n_q + 3) // 4):
                    qs = list(range(4 * hf, min(4 * hf + 4, n_q)))
                    # V = block-diag W @ L, two quarters per PSUM tile
                    vps = []  # (vp_tile, [quarters], col0_in_lr)
                    for g0 in range(0, len(qs), 2):
                        gq = qs[g0 : g0 + 2]
                        vp = vpool.tile(
                            [128, CHUNK * len(gq)], F32, tag=f"v{len(gq)}", name="v_ps"
                        )
                        for t, h2 in enumerate(gq):
                            nc.tensor.matmul(
                                vp[:, CHUNK * t : CHUNK * (t + 1)],
                                wdiag[:, 128 * var_of[u][h2] : 128 * (var_of[u][h2] + 1)],
                                lr[:, CHUNK * h2 : CHUNK * (h2 + 1)],
                                tile_position=(0, 0),
                            )
                        vps.append((vp, gq))

                    if len(pending) > 1:
                        flush(pending.pop(0), hp_idx)
                        hp_idx += 1

                    # Z = V (*) R elementwise, fp16. DVE's 1x PSUM path is
                    # the kernel's scarcest resource, so alternate: pair A
                    # multiplies straight from PSUM (DVE 1x); pair B is
                    # cast-copied PSUM->SBUF by ACT (which sits next to
                    # PSUM), letting DVE run it in 2x fp16 SBUF mode.
                    z_parts = []
                    for idx, (vp, gq) in enumerate(vps):
                        w = CHUNK * len(gq)
                        src_v = vp
                        if idx % 2 == 1:
                            vsb = vbp.tile(
                                [128, w], F16, tag=f"vs{len(gq)}", name="v_sb"
                            )
                            nc.scalar.copy(vsb[:], vp[:])
                            src_v = vsb
                        z = zp.tile([128, w], F16, tag=f"z{len(gq)}", name="z_sb")
                        nc.vector.tensor_tensor(
                            z[:],
                            src_v[:],
                            lr[:, fcols + CHUNK * gq[0] : fcols + CHUNK * gq[0] + w],
                            op=mybir.AluOpType.mult,
                        )
                        z_parts.append((z, 0, gq[0] - 4 * hf, len(gq)))
                    cur_pair.append((z_parts, h_idx % 2))
                    h_idx += 1
                    if len(cur_pair) == 2:
                        pending.append(cur_pair)
                        cur_pair = []

            if cur_pair:
                pending.append(cur_pair)
            for p in pending:
                flush(p, hp_idx)
                hp_idx += 1
    nc.finalize()
    return nc


def _relayout(X, n_q):
    """[2048*n_q, 32] edge-major -> [128, 512*n_q] PE block layout:
    [32*pb + k, 512*h2 + n] = X[512*n_q*pb + 512*h2 + n, k]."""
    return (
        X.reshape(4, n_q, CHUNK, DIM).transpose(0, 3, 1, 2).reshape(128, n_q * CHUNK)
    )


def _prep_inputs(left, right, rid):
    """Sort/pad/shard/relayout on the host. Returns device arrays + recovery info."""
    E = left.shape[0]
    perm = np.argsort(rid, kind="stable")
    counts = np.bincount(rid, minlength=NUM_REL).astype(np.int64)
    starts = np.zeros(NUM_REL + 1, dtype=np.int64)
    np.cumsum(counts, out=starts[1:])

    # per-core segment length per relation, multiple of CHUNK
    seg = [
        int(math.ceil(c / (N_CORES * CHUNK))) * CHUNK if c > 0 else 0 for c in counts
    ]
    per_core_real = int(sum(seg))
    n_chunks = max(1, per_core_real // CHUNK)
    units, n_full, q_t = _unit_list(n_chunks)
    T = 512 * 4 * sum(units)  # padded per-core slot count

    # relation schedule of each sorted 512-chunk (identical on every core)
    rel_sched = []
    for r in range(NUM_REL):
        rel_sched += [r] * (seg[r] // CHUNK)
    rel_sched += [0] * (T // CHUNK - len(rel_sched))

    # gather index (into sorted order) for each device slot; -1 = padding
    gidx = np.full((N_CORES, T), -1, dtype=np.int64)
    off = 0
    for r in range(NUM_REL):
        s = seg[r]
        if s == 0:
            continue
        ar = np.arange(s, dtype=np.int64)
        for c in range(N_CORES):
            src = c * s + ar
            gidx[c, off : off + s] = np.where(src < counts[r], starts[r] + src, -1)
        off += s

    L_s = left[perm]
    R_s = right[perm]

    LR = (
        np.zeros((N_CORES, n_full, 128, 2 * TILE_FREE), np.float16)
        if n_full
        else None
    )
    LRT = (
        np.zeros((N_CORES, 128, 2 * q_t * CHUNK), np.float16) if q_t else None
    )
    for c in range(N_CORES):
        gi = gidx[c]
        msk = gi >= 0
        Lc = np.zeros((T, DIM), np.float32)
        Rc = np.zeros((T, DIM), np.float32)
        Lc[msk] = L_s[gi[msk]]
        Rc[msk] = R_s[gi[msk]]
        Lc = Lc.astype(np.float16)
        Rc = Rc.astype(np.float16)
        full_e = n_full * TILE_E
        if n_full:
            Lf = Lc[:full_e].reshape(n_full, TILE_E, DIM)
            Rf = Rc[:full_e].reshape(n_full, TILE_E, DIM)
            for u in range(n_full):
                LR[c, u, :, :TILE_FREE] = _relayout(Lf[u], N_Q)
                LR[c, u, :, TILE_FREE:] = _relayout(Rf[u], N_Q)
        if q_t:
            LRT[c, :, : q_t * CHUNK] = _relayout(Lc[full_e:], q_t)
            LRT[c, :, q_t * CHUNK :] = _relayout(Rc[full_e:], q_t)
    return perm, gidx, units, tuple(rel_sched), LR, LRT


def _recover_scores(results, perm, gidx, units, E):
    n_full = sum(1 for q in units if q == N_Q)
    q_t = 0 if len(units) == n_full else units[-1]
    n_halves = 2 * n_full + ((q_t + 3) // 4 if q_t else 0)
    n_og = math.ceil(n_halves / GROUP_HALVES)
    T = 512 * 4 * sum(units)

    scores_sorted = np.zeros(E, np.float32)
    for c in range(N_CORES):
        sc = np.asarray(results[c]["scores"], dtype=np.float32)
        # [og, 32*(pp%4) + 16*par + r, 512*(pp//4) + n];
        # half = 2*(4*colblk + pblk4) + par
        sc = (
            sc.reshape(n_og, 4, 2, 16, 4, CHUNK)
            .transpose(0, 4, 1, 2, 3, 5)
            .reshape(n_og * GROUP_HALVES, 16, CHUNK)[:n_halves]
        )
        out = np.empty(T, np.float32)
        if n_full:
            # half h=2u+hf; row r=4j+b; pos = 16384u + 4096b + 2048hf + 512j + n
            out[: n_full * TILE_E] = (
                sc[: 2 * n_full]
                .reshape(n_full, 2, 4, 4, CHUNK)
                .transpose(0, 3, 1, 2, 4)
                .reshape(n_full * TILE_E)
            )
        if q_t:
            base = n_full * TILE_E
            for t in range((q_t + 3) // 4):
                half = sc[2 * n_full + t]
                for j in range(min(4, q_t - 4 * t)):
                    for b in range(4):
                        pos = base + CHUNK * (q_t * b + 4 * t + j)
                        out[pos : pos + CHUNK] = half[4 * j + b]
        gi = gidx[c]
        msk = gi >= 0
        scores_sorted[gi[msk]] = out[msk]
    scores = np.empty(E, np.float32)
    scores[perm] = scores_sorted
    return scores


def kernel(left_emb, right_emb, r_id, W):
    global LAST_RESULTS
    left = np.ascontiguousarray(np.asarray(left_emb, dtype=np.float32))
    right = np.ascontiguousarray(np.asarray(right_emb, dtype=np.float32))
    rid = np.asarray(r_id).astype(np.int64)
    Wn = np.asarray(W, dtype=np.float32)
    E = left.shape[0]

    perm, gidx, units, rel_sched, LR, LRT = _prep_inputs(left, right, rid)

    # block-diagonal stationary-W variants, one per distinct quarter combo
    combos, _ = _quarter_variants(units, rel_sched)
    wdiag3 = np.zeros((len(combos), 128, 128), np.float32)
    for v, combo in enumerate(combos):
        for pb, r in enumerate(combo):
            wdiag3[v, 32 * pb : 32 * pb + 32, 32 * pb : 32 * pb + 32] = Wn[r]
    # packed [128, n_var*128] so the whole table loads in one contiguous DMA
    wdiag = np.ascontiguousarray(
        wdiag3.transpose(1, 0, 2).reshape(128, len(combos) * 128)
    ).astype(np.float16)
    # shifted block-ones: slice s=4p+j sums 32-row block b of quarter j
    # into dense row 16p + 4j + b of the [32,512] pair-score tile
    ones32 = np.zeros((128, 256), np.float16)
    for p in range(2):
        for j in range(4):
            for b in range(4):
                ones32[32 * b : 32 * b + 32, 32 * (4 * p + j) + 16 * p + 4 * j + b] = 1.0

    key = (tuple(units), rel_sched)
    if key not in _module_cache:
        _module_cache.clear()
        _module_cache[key] = _build_module(list(units), rel_sched)
    nc = _module_cache[key]

    in_maps = []
    for c in range(N_CORES):
        m = {"wdiag": wdiag, "ones32": ones32}
        if LR is not None:
            m["lr"] = LR[c]
        if LRT is not None:
            m["lrt"] = LRT[c]
        in_maps.append(m)
    trace = bool(int(os.environ.get("KERNEL_TRACE", "0")))
    kwargs = {}
    if trace:
        _ensure_profiling_hooks()
        tdir = os.environ.get("KERNEL_TRACE_DIR")
        if tdir:
            os.makedirs(tdir, exist_ok=True)
            kwargs["tmpdir"] = tdir
    res = bass_utils.run_bass_kernel_spmd(
        nc, in_maps, core_ids=list(range(N_CORES)), trace=trace, **kwargs
    )
    LAST_RESULTS = res
    return _recover_scores(res.results, perm, gidx, units, E)
